# revision 32
# baseline (speedup 1.0000x reference)
"""Trainium2 Bass kernel for BilinearInteraction.

out[b, p] = x[b, i_p, :] @ W[p] @ x[b, j_p, :]  for the 780 field pairs
(i, j), i < j, of F=40 fields (row-major triu order).

Architecture (8 NeuronCores, data-parallel over batch, B_loc=256):
  - "b-T" stage-1 layout: PE matmuls produce Y[(pair, e), b] in PSUM
    (pairs x e on partitions, batch on the free dim).
  - Tiles: one [128, 256] PSUM slice holds 2 pairs sharing one i-field:
    (i, 2t) and (i, 2t+1), matching xT chunk t (fields 2t / 2t+1 on the
    two partition halves). W is host-permuted (bf16) into per-tile
    contiguous lhsT blocks (zero blocks for invalid (i==2t, 2t) slots).
    4 tiles of one chunk share a 2-bank [128, 1024] PSUM group.
  - Host pre-transposes x into the two layouts the kernel needs
    (xtcb bf16 for the multiply, xtlo bf16 for stage-1 rhs).
  - stage 1: PE matmul Y = Wtile.T @ xT_i  (bf16, K=64, M=128, N=256).
  - stage 2 (three paths, weighted round-robin to balance engines):
      A: ACT evicts Y -> bf16 SBUF; DVE multiplies by the xtcb chunk at
         the 2x packed rate -> z bf16.
      F: DVE multiplies straight from PSUM f32 (1x) -> z bf16.
      P: Pool (GPSIMD) multiplies straight from PSUM f32 -> z bf16.
  - stage 3: reversed-operand PE matmuls: each z half-tile [128, 128]
    is the *stationary* operand, the moving operand is a fixed [128, 2]
    ones mask (col 0 selects partitions 0-63 = pair A, col 1 selects
    64-127 = pair B), producing out[(b-half), 2 pairs] f32. The N=2
    moving stream makes each reduce matmul ~2 PE cycles; the weight
    load rides the PE's weight-load path. 128 tiles accumulate into one
    [128, 512] PSUM bank (4 cols/tile: h0qA h0qB h1qA h1qB); the bank
    is evicted (ACT/DVE alternating) and DMA'd to outT2[128, epoch*512]
    once per 32 groups. The host inverse-permutes (tile, half, slot)
    columns into (batch, pair) and concatenates the batch shards.
  - DMA queues: W batches + first x pieces on SP/ACT HWDGE queues, x
    tails on Pool SWDGE before the compute loop leans on Pool.
"""

import numpy as np
import ml_dtypes

import concourse.bass as bass
import concourse.mybir as mybir
import concourse.tile as tile
from concourse import bacc
from concourse.bass_utils import run_bass_kernel_spmd

B, F, D = 2048, 40, 64
P = F * (F - 1) // 2  # 780
NCORES = 8
B_LOC = B // NCORES  # 256
F32 = mybir.dt.float32
BF16 = mybir.dt.bfloat16

NCHUNK = F // 2  # 20 xT chunks (2 fields each)
# tile list: (t, i) — pairs (i, 2t) [dummy if i==2t] and (i, 2t+1)
TILES = [(t, i) for t in range(NCHUNK) for i in range(2 * t + 1)]
NTILES = len(TILES)  # 400

GMAX = 4  # tiles per PSUM group (4 x 256 cols = 2 banks)
REDUCE_DELAY = 12  # groups of reduce-matmul lag (software pipelining)
MULT_DELAY = 6  # groups of lag between evict and SBUF multiply (A/G paths)

TILES_PER_RB = 128  # tiles per reduce PSUM bank (4 f32 cols per tile)
EPOCH_ENDS = [128, 256, 384, NTILES]
NEPOCH = len(EPOCH_ENDS)
OUT_COLS = NTILES * 4  # 1600
_E_START = {}
_E_IDX = {}
_prev = 0
for _e, _end in enumerate(EPOCH_ENDS):
    for _k in range(_prev, _end):
        _E_START[_k] = _prev
        _E_IDX[_k] = _e
    _prev = _end

# stage-2 path schedule: (A) ACT evict + DVE mult, (F) DVE fused mult
# from PSUM, (G) ACT evict + Pool mult from SBUF (GPSIMD cannot touch
# PSUM). Weights tuned so ACT / DVE / Pool all sit just under the drain
# cadence: ACT = 1038(a+g), DVE = 593a + 1192f, Pool = 2126g.
PATH_WEIGHTS = {"A": 0.31, "F": 0.39, "G": 0.30}
POOL_SKIP_HEAD = 0  # first groups avoid G (Pool is busy with x-tail DMA gen)

# DMA prefetch slicing (units: xtlo = fields, xtc/xtcb = chunks). The first
# pieces are small so the first stage-1/mult tiles unblock early; x DMAs ride
# the ACT HWDGE queue so they overlap W DMAs issued on the SP queue.
XTLO_PIECES = [(0, 4), (4, 10), (10, 16), (16, 28), (28, 40)]
XCHUNK_PIECES = [(0, 2), (2, 6), (6, 12), (12, 20)]
WDMA_PREFETCH = 6  # W batches issued before the compute loop

WDMA_BATCH = 16  # stage-1 lhsT tiles per DMA


def _build_groups():
    # uniform cross-chunk groups: cut the tile stream every GMAX tiles.
    # A group is a list of (chunk, ilist) segments; chunk boundaries fall
    # mid-group, so the multiply runs one TT per segment while the evict
    # covers the whole group.
    groups = []
    cur = []
    tot = 0
    for t in range(NCHUNK):
        for i in range(2 * t + 1):
            if cur and cur[-1][0] == t:
                cur[-1][1].append(i)
            else:
                cur.append((t, [i]))
            tot += 1
            if tot == GMAX:
                groups.append(cur)
                cur = []
                tot = 0
    if cur:
        groups.append(cur)
    return groups


GROUPS = _build_groups()
# split the final group so the end-of-kernel drain chain (evict -> mult ->
# reduce -> ob -> DMA) finishes on a minimal single-tile group
_segs = GROUPS[-1]
_tl, _ill = _segs[-1]
if len(_ill) > 1:
    GROUPS[-1] = _segs[:-1] + [(_tl, _ill[:-1])]
    GROUPS.append([(_tl, _ill[-1:])])
elif len(_segs) > 1:
    GROUPS[-1] = _segs[:-1]
    GROUPS.append([(_tl, _ill)])


F_HEAD = 0  # W-DMA-limited head: F has the shortest dependency chain
F_TAIL = 0  # drain tail: keep the last groups off the 2-hop paths


def _build_path_schedule(ngroups):
    """Error-diffusion weighted round-robin over stage-2 paths."""
    acc = {k: 0.0 for k in PATH_WEIGHTS}
    out = []
    for g in range(ngroups):
        for k in PATH_WEIGHTS:
            acc[k] += PATH_WEIGHTS[k]
        if g < F_HEAD or g >= ngroups - F_TAIL:
            out.append("F")
            acc["F"] -= 1.0
            continue
        # choose the path with the largest accumulated credit, with
        # restrictions for the head (Pool busy with DMA gen)
        cand = sorted(acc, key=lambda k: -acc[k])
        pick = None
        for k in cand:
            if k == "G" and g < POOL_SKIP_HEAD:
                continue
            pick = k
            break
        acc[pick] -= 1.0
        out.append(pick)
    return out


PATH_SCHED = _build_path_schedule(len(GROUPS))


def host_prep(W: np.ndarray):
    """Build Wt3 [64, NTILES*128] bf16 and the output column permutation."""
    # Wt2[d, p, e]
    Wt2 = np.ascontiguousarray(W.transpose(1, 0, 2))  # [64, 780, 64]
    pair_idx = -np.ones((F, F), dtype=np.int64)
    k = 0
    for i in range(F):
        for j in range(i + 1, F):
            pair_idx[i, j] = k
            k += 1
    Wt3 = np.zeros((D, NTILES * 128), dtype=np.float32)
    rows = []
    for k, (t, i) in enumerate(TILES):
        jA, jB = 2 * t, 2 * t + 1
        pA = pair_idx[i, jA] if i < jA else -1
        pB = pair_idx[i, jB]
        if pA >= 0:
            Wt3[:, k * 128 : k * 128 + 64] = Wt2[:, pA, :]
        Wt3[:, k * 128 + 64 : k * 128 + 128] = Wt2[:, pB, :]
        rows.append((pA, pB))
    # outT2 col of (tile k, half h, slot q) = 4k + 2h + q.
    # For pair p: ks[p] = tile, qs[p] = slot.
    ks = np.zeros(P, dtype=np.int64)
    qs = np.zeros(P, dtype=np.int64)
    for k, (pA, pB) in enumerate(rows):
        if pA >= 0:
            ks[pA] = k
            qs[pA] = 0
        ks[pB] = k
        qs[pB] = 1
    return Wt3.astype(ml_dtypes.bfloat16), ks, qs


def build_nc():
    nc = bacc.Bacc("TRN2", target_bir_lowering=False, debug=False)

    xtlo_dram = nc.dram_tensor(
        "xtlo", [64, F * B_LOC], BF16, kind="ExternalInput"
    ).ap()
    xtcb_dram = nc.dram_tensor(
        "xtcb", [128, NCHUNK * B_LOC], BF16, kind="ExternalInput"
    ).ap()
    wt_dram = nc.dram_tensor("Wt3", [D, NTILES * 128], BF16, kind="ExternalInput").ap()
    out_dram = nc.dram_tensor("outT2", [128, OUT_COLS], F32, kind="ExternalOutput").ap()

    with tile.TileContext(nc) as tc:
        with (
            tc.tile_pool(name="persist", bufs=1) as persist,
            tc.tile_pool(name="wpool", bufs=6) as wpool,
            tc.tile_pool(name="zpool", bufs=14) as zpool,
            tc.tile_pool(name="ybfpool", bufs=12) as ybfpool,
            tc.tile_pool(name="opool", bufs=2) as opool,
            tc.tile_pool(name="ypsum", bufs=3, space=bass.MemorySpace.PSUM) as ypsum,
            tc.tile_pool(name="rpsum", bufs=2, space=bass.MemorySpace.PSUM) as rpsum,
        ):
            # [128, 2] ones mask for the reversed e-reduction: col 0 hits
            # partitions 0-63 (pair A), col 1 hits 64-127 (pair B).
            ones2 = persist.tile([128, 2], BF16, tag="ones2")
            nc.vector.memset(ones2[:, :], 0.0)
            nc.vector.memset(ones2[0:64, 0:1], 1.0)
            nc.vector.memset(ones2[64:128, 1:2], 1.0)

            # XTC[(f%2)*64 + d, t*256 + m*128 + b]  (f = 2t + f%2) and the
            # low-half layout (all fields at partitions 0-63) are both
            # pre-transposed on the host and DMA'd directly.
            xtlo = persist.tile([64, F * B_LOC], BF16, tag="xtlo")
            xtcb = persist.tile([128, NCHUNK * B_LOC], BF16, tag="xtcb")

            # W batches are DMA'd on the SP queue; x layouts ride the ACT
            # HWDGE queue / Pool SWDGE so the descriptor streams overlap.
            wtiles = []

            def w_dma(bi):
                kt0 = bi * WDMA_BATCH
                nw = min(WDMA_BATCH, NTILES - kt0)
                wt = wpool.tile([64, WDMA_BATCH * 128], BF16, tag="w")
                if bi == 0:
                    # two half-DMAs: tiles 0-7 land ~0.4us sooner
                    for lo, hi in ((0, 8), (8, nw)):
                        nc.sync.dma_start(
                            out=wt[:, lo * 128 : hi * 128],
                            in_=wt_dram[:, lo * 128 : hi * 128],
                        )
                else:
                    nc.sync.dma_start(
                        out=wt[:, : nw * 128],
                        in_=wt_dram[:, kt0 * 128 : (kt0 + nw) * 128],
                    )
                wtiles.append(wt)

            def x_dma(piece, eng):
                kind, lo, hi = piece
                src, dst = {
                    "xtlo": (xtlo_dram, xtlo),
                    "xtcb": (xtcb_dram, xtcb),
                }[kind]
                eng.dma_start(
                    out=dst[:, lo * B_LOC : hi * B_LOC],
                    in_=src[:, lo * B_LOC : hi * B_LOC],
                )

            w_dma(0)
            x_dma(("xtlo",) + XTLO_PIECES[0], nc.scalar)
            x_dma(("xtcb",) + XCHUNK_PIECES[0], nc.gpsimd)
            x_dma(("xtlo",) + XTLO_PIECES[1], nc.scalar)
            w_dma(1)
            x_dma(("xtcb",) + XCHUNK_PIECES[1], nc.gpsimd)
            x_dma(("xtlo",) + XTLO_PIECES[2], nc.scalar)
            w_dma(2)
            w_dma(3)
            w_dma(4)
            w_dma(5)
            x_dma(("xtcb",) + XCHUNK_PIECES[2], nc.gpsimd)
            x_dma(("xtlo",) + XTLO_PIECES[3], nc.sync)
            x_dma(("xtcb",) + XCHUNK_PIECES[3], nc.sync)
            x_dma(("xtlo",) + XTLO_PIECES[4], nc.sync)
            DEFERRED_X = {}

            rbs = [None]
            k = 0

            def emit_reduce(z, k0, gsz):
                # reversed-operand reduce: z half-tile stationary, ones2
                # moving (N=2). Accumulates nothing — each tile owns its 4
                # output columns in the epoch's PSUM bank.
                for idx in range(gsz):
                    kt = k0 + idx
                    est = _E_START[kt]
                    s = kt - est
                    if s == 0:
                        rbs[0] = rpsum.tile([128, 512], F32, tag="rb", name="rb")
                    rb = rbs[0]
                    for h in range(2):
                        nc.tensor.matmul(
                            rb[:, s * 4 + h * 2 : s * 4 + h * 2 + 2],
                            z[:, idx * B_LOC + h * 128 : idx * B_LOC + (h + 1) * 128],
                            ones2[:, :],
                            start=True,
                            stop=True,
                        )
                    epoch = _E_IDX[kt]
                    if kt == EPOCH_ENDS[epoch] - 1:
                        ncols = (s + 1) * 4
                        ob = opool.tile([128, 512], F32, tag="ob")
                        # alternate the epoch eviction between DVE and ACT
                        if epoch % 2 == 1:
                            nc.vector.tensor_copy(
                                out=ob[:, :ncols], in_=rb[:, :ncols]
                            )
                        else:
                            nc.scalar.copy(out=ob[:, :ncols], in_=rb[:, :ncols])
                        nc.sync.dma_start(
                            out=out_dram[
                                :, est * 4 : est * 4 + ncols
                            ],
                            in_=ob[:, :ncols],
                        )

            # per-group pipeline records: z filled in at mult-emission time
            entries = []  # dicts: k0, gsz, segs, src (ybf or y), path, z
            rptr = [0]
            mptr = [0]

            def emit_mult(ent):
                # stage 2: z = Y * xT[j-fields chunk t]  (bf16 out), one TT
                # per chunk segment with stride-0 broadcast of the xtc chunk
                z = zpool.tile([128, GMAX * B_LOC], BF16, tag="z")
                src_tile = ent["src"]
                off = 0
                for t, ilist in ent["segs"]:
                    n = len(ilist)
                    in1 = xtcb[
                        :, None, t * B_LOC : (t + 1) * B_LOC
                    ].to_broadcast([128, n, B_LOC])
                    zs = z[:, off * B_LOC : (off + n) * B_LOC].rearrange(
                        "p (n b) -> p n b", n=n
                    )
                    ss = src_tile[
                        :, off * B_LOC : (off + n) * B_LOC
                    ].rearrange("p (n b) -> p n b", n=n)
                    meng = nc.gpsimd if ent["path"] in ("G", "H") else nc.vector
                    meng.tensor_tensor(zs, ss, in1, mybir.AluOpType.mult)
                    off += n
                ent["z"] = z

            def pump_reduces(limit):
                # emit reduces strictly in tile order, only for groups whose
                # z exists and that are at least REDUCE_DELAY groups old
                while rptr[0] < limit and entries[rptr[0]]["z"] is not None:
                    ent = entries[rptr[0]]
                    emit_reduce(ent["z"], ent["k0"], ent["gsz"])
                    rptr[0] += 1

            for gidx, segs in enumerate(GROUPS):
                if gidx in DEFERRED_X:
                    x_dma(DEFERRED_X[gidx], nc.sync)
                gsz = sum(len(il) for _, il in segs)
                # stage 1: Y[(p, e), b] = Wtile.T @ xT_i — gsz tiles share
                # one PSUM bank pair (disjoint column quarters)
                y = ypsum.tile([128, GMAX * B_LOC], F32, tag="y")
                idx = 0
                for t, ilist in segs:
                    for i in ilist:
                        kt = k + idx
                        if kt % WDMA_BATCH == 0:
                            bi = kt // WDMA_BATCH + 3  # three batches of lead
                            if WDMA_PREFETCH <= bi < (NTILES + WDMA_BATCH - 1) // WDMA_BATCH:
                                w_dma(bi)
                        kk = kt % WDMA_BATCH
                        wchunk = wtiles[kt // WDMA_BATCH]
                        nc.tensor.matmul(
                            y[:, idx * B_LOC : (idx + 1) * B_LOC],
                            wchunk[:, kk * 128 : (kk + 1) * 128],
                            xtlo[:, i * B_LOC : (i + 1) * B_LOC],
                            start=True,
                            stop=True,
                        )
                        idx += 1

                path = PATH_SCHED[gidx] if gsz > 1 else "A"
                ent = {"k0": k, "gsz": gsz, "segs": segs, "path": path, "z": None}
                entries.append(ent)
                if path in ("A", "G", "H"):
                    # ACT (A/G) or DVE (H) evicts PSUM -> bf16 now (frees
                    # the ypsum bank); the SBUF multiply runs MULT_DELAY
                    # groups later so the mult engine never waits on a
                    # just-issued evict.
                    ybf = ybfpool.tile([128, GMAX * B_LOC], BF16, tag="ybf")
                    if path == "H":
                        nc.vector.tensor_copy(
                            out=ybf[:, : gsz * B_LOC], in_=y[:, : gsz * B_LOC]
                        )
                    else:
                        nc.scalar.copy(
                            out=ybf[:, : gsz * B_LOC], in_=y[:, : gsz * B_LOC]
                        )
                    ent["src"] = ybf
                else:  # "F": DVE multiplies straight from PSUM (fused evict)
                    ent["src"] = y
                    emit_mult(ent)

                # delayed SBUF multiplies for A/G groups
                while mptr[0] <= gidx - MULT_DELAY:
                    if entries[mptr[0]]["z"] is None:
                        emit_mult(entries[mptr[0]])
                    mptr[0] += 1
                # stage 3 trails by REDUCE_DELAY groups, in strict order
                pump_reduces(max(0, gidx + 1 - REDUCE_DELAY))
                k += gsz

            for ent in entries:
                if ent["z"] is None:
                    emit_mult(ent)
            pump_reduces(len(entries))

    nc.compile()
    return nc


_NC = None


def kernel(x: np.ndarray, W: np.ndarray) -> np.ndarray:
    global _NC
    x = np.ascontiguousarray(np.asarray(x, dtype=np.float32))
    W = np.ascontiguousarray(np.asarray(W, dtype=np.float32))
    assert x.shape == (B, F, D) and W.shape == (P, D, D)

    Wt3, ks, qs = host_prep(W)

    if _NC is None:
        _NC = build_nc()

    in_maps = []
    for c in range(NCORES):
        xs = x[c * B_LOC : (c + 1) * B_LOC]  # [256, 40, 64]
        v = xs.transpose(1, 2, 0).reshape(NCHUNK, 2, D, B_LOC)
        xtc = np.ascontiguousarray(
            v.transpose(1, 2, 0, 3).reshape(128, NCHUNK * B_LOC)
        )
        xtlo = np.ascontiguousarray(
            xs.transpose(2, 1, 0).reshape(D, F * B_LOC)
        ).astype(ml_dtypes.bfloat16)
        xtcb = xtc.astype(ml_dtypes.bfloat16)
        in_maps.append({"xtcb": xtcb, "xtlo": xtlo, "Wt3": Wt3})
    res = run_bass_kernel_spmd(_NC, in_maps, core_ids=list(range(NCORES)))
    out = np.empty((B, P), dtype=np.float32)
    for c in range(NCORES):
        outT2 = res.results[c]["outT2"]  # [128, OUT_COLS]
        arr = outT2.reshape(128, NTILES, 2, 2)
        # res[p, b, h] = arr[b, ks[p], h, qs[p]]
        sel = arr[:, ks, :, qs]  # [P, 128, 2]
        out[c * B_LOC : (c + 1) * B_LOC, :] = sel.transpose(2, 1, 0).reshape(
            2 * 128, P
        )
    return out
